# revision 37
# baseline (speedup 1.0000x reference)
"""Trainium2 Bass kernel for nn_Decoder_Layer_53738630807778.

8-core data parallel over B=2048.  Feature-major on device (feature dim on
SBUF partitions, tokens on the free axis); host pre-transposes and pre-adds
role_embeds.  Big GEMMs (QKV/O projections, FFNs, aggregations) run in
fp8e4 with DoubleRow perf mode (two 128-row k-subtiles per matmul) and
fp32 PSUM; weights are pre-scaled x64 on the host (descale folds into the
PSUM->SBUF copies / activations), except the FFN second layers which stay
unscaled so the residual add can read PSUM directly.

Attention (L=6, H=16, hd=64) per (set g, batch-slab of 128):
  scores = DVE q*k elementwise -> PE block-ones matmul reduces each head's
           64 partition rows (scaled 1/8); softmax on [16 x L*128].
  alpha  -> expanded back to feature rows with a (16,128) selection matmul,
           staged to SBUF bf16 so the AV mul runs in DVE 2x mode.
  AV     = DVE mul + strided reduce over the 6 keys.

ln1/ln3 have identity affine and every bias is zero (asserted), so they
fold: LN scale-invariance + relu positive homogeneity kill the rstd factor
(ln2/ln4 renormalize); the mean is subtracted explicitly from the FFN input
(PE row-broadcast of the ones-matmul row mean).  Verb sets g=1..5 are
processed first so the noun-path aggregation overlaps g=0's attention.
"""

import sys
import numpy as np

if "/opt/trn_rl_repo" not in sys.path:
    sys.path.insert(0, "/opt/trn_rl_repo")

import ml_dtypes

BF = ml_dtypes.bfloat16
F8 = ml_dtypes.float8_e4m3

D = 1024
H = 16
DFF = 4096
S = 5
L = 6
G = 6
NCORES = 8
NB = D // 128
NF = DFF // 128
EPS = 1e-5
WS = 64.0          # fp8 weight pre-scale
SLAB = 128

_cache = {}


def _chunks(n, step=512):
    out = []
    off = 0
    while off < n:
        out.append((off, min(step, n - off)))
        off += step
    return out


def build(bc):
    import concourse.bacc as bacc
    import concourse.mybir as mybir
    import concourse.tile as tile

    F32 = mybir.dt.float32
    BF16 = mybir.dt.bfloat16
    FP8 = mybir.dt.float8e4
    AF = mybir.ActivationFunctionType
    ALU = mybir.AluOpType
    AX = mybir.AxisListType
    DR = mybir.MatmulPerfMode.DoubleRow

    NSLAB = bc // SLAB
    LBS = L * SLAB

    nc = bacc.Bacc("TRN2", target_bir_lowering=False, debug=False)

    src_d = nc.dram_tensor("src", [128, G, NSLAB, NB, LBS], FP8, kind="ExternalInput")
    tgt_d = nc.dram_tensor("tgt", [128, NB, L, bc], BF16, kind="ExternalInput")
    wq_d = nc.dram_tensor("wq", [128, NB, D], FP8, kind="ExternalInput")
    wk_d = nc.dram_tensor("wk", [128, NB, D], FP8, kind="ExternalInput")
    wv_d = nc.dram_tensor("wv", [128, NB, D], FP8, kind="ExternalInput")
    wo_d = nc.dram_tensor("wo", [128, NB, D], FP8, kind="ExternalInput")
    w11_d = nc.dram_tensor("w11", [NB, 128, DFF], BF16, kind="ExternalInput")
    w12_d = nc.dram_tensor("w12", [NF, 128, D], BF16, kind="ExternalInput")
    w21_d = nc.dram_tensor("w21", [NB, 128, DFF], BF16, kind="ExternalInput")
    w22_d = nc.dram_tensor("w22", [NF, 128, D], BF16, kind="ExternalInput")
    ag1_d = nc.dram_tensor("ag1", [128, S * NB, D], FP8, kind="ExternalInput")
    ag2_d = nc.dram_tensor("ag2", [128, S * NB, D], FP8, kind="ExternalInput")
    ones_d = nc.dram_tensor("onesb", [NB, 128, H], BF16, kind="ExternalInput")
    out_d = nc.dram_tensor("out_t", [NB, 128, L, bc], F32, kind="ExternalOutput")

    IWS = 1.0 / WS

    with tile.TileContext(nc) as tc:
        with tc.tile_pool(name="glob", bufs=1) as glob:
            msgs_v = glob.tile([128, S * NB, bc], FP8, tag="msv", name="msv")
            msgs_n = glob.tile([128, S * NB, bc], FP8, tag="msn", name="msn")
            gates_v = [glob.tile([128, bc], BF16, tag=f"gv{o}", name=f"gv{o}")
                       for o in range(NB)]
            gates_n = [glob.tile([128, bc], BF16, tag=f"gn{o}", name=f"gn{o}")
                       for o in range(NB)]
            onescol = glob.tile([128, 1], BF16, tag="onescol", name="onescol")
            onesrow = glob.tile([1, 128], BF16, tag="onesrow", name="onesrow")
            onesrow32 = glob.tile([1, 128], F32, tag="onesrow32", name="onesrow32")
            epst = glob.tile([1, 1], F32, tag="epst", name="epst")
            nc.gpsimd.memset(onescol[:], 1.0 / 1024.0)
            nc.gpsimd.memset(onesrow[:], 1.0)
            nc.gpsimd.memset(onesrow32[:], 1.0)
            nc.gpsimd.memset(epst[:], EPS)

            # ================= PASS A: attention =================
            with tc.tile_pool(name="wa", bufs=1) as wa, \
                 tc.tile_pool(name="suba", bufs=2) as suba, \
                 tc.tile_pool(name="subk", bufs=2) as subk, \
                 tc.tile_pool(name="subq", bufs=2) as subq, \
                 tc.tile_pool(name="prodp", bufs=3) as prodp, \
                 tc.tile_pool(name="palp", bufs=2) as palp, \
                 tc.tile_pool(name="avbp", bufs=1) as avbp, \
                 tc.tile_pool(name="smallp", bufs=2) as smallp, \
                 tc.tile_pool(name="alsc", bufs=2, space="DRAM") as alscp, \
                 tc.tile_pool(name="psmm", bufs=3, space="PSUM") as psmm, \
                 tc.tile_pool(name="pssc", bufs=1, space="PSUM") as pssc:

                wq = wa.tile([128, NB, D], FP8, tag="wq", name="wq")
                wk = wa.tile([128, NB, D], FP8, tag="wk", name="wk")
                wv = wa.tile([128, NB, D], FP8, tag="wv", name="wv")
                wo = wa.tile([128, NB, D], FP8, tag="wo", name="wo")
                onesb = [wa.tile([128, H], BF16, tag=f"ones{i}", name=f"ones{i}")
                         for i in range(NB)]
                nc.sync.dma_start(wq[:], wq_d[:])
                nc.sync.dma_start(wk[:], wk_d[:])
                nc.sync.dma_start(wv[:], wv_d[:])
                nc.sync.dma_start(wo[:], wo_d[:])
                for i in range(NB):
                    nc.sync.dma_start(onesb[i][:], ones_d[i])

                def proj(wt, dst, ssrc, qoff, ntok):
                    # both N-chunks of one output block go into a single
                    # 2-bank PSUM tile -> one fused descale copy per block
                    for o in range(NB):
                        ps = psmm.tile([128, 1024], F32, tag="mm", name="mm")
                        for off, ln in _chunks(ntok):
                            for k in range(4):
                                nc.tensor.matmul(
                                    ps[:, off:off + ln],
                                    wt[:, 2 * k:2 * k + 2, o * 128:(o + 1) * 128],
                                    ssrc[:, 2 * k:2 * k + 2,
                                         qoff + off:qoff + off + ln],
                                    start=(k == 0), stop=(k == 3),
                                    perf_mode=DR)
                        nc.scalar.activation(dst[:, o, 0:ntok],
                                             ps[:, :ntok], AF.Copy, scale=IWS)

                def emit_kvq(g, slab):
                    ssrc = suba.tile([128, NB, LBS], FP8, tag="ssrc", name="ssrc")
                    nc.sync.dma_start(ssrc[:], src_d[:, g, slab])
                    tk = subk.tile([128, NB, LBS], BF16, tag="tk", name="tk")
                    tv = subk.tile([128, NB, LBS], BF16, tag="tv", name="tv")
                    proj(wk, tk, ssrc, 0, LBS)
                    proj(wv, tv, ssrc, 0, LBS)
                    nq = S if g == 0 else 1
                    qoff = SLAB if g == 0 else 0
                    nqt = nq * SLAB
                    tq = subq.tile([128, NB, S * SLAB], BF16, tag="tq", name="tq")
                    proj(wq, tq, ssrc, qoff, nqt)
                    return (g, slab, nq, nqt, tk, tv, tq)

                def emit_attn(stt):
                    g, slab, nq, nqt, tk, tv, tq = stt
                    tao = [subq.tile([128, 4, S * SLAB], FP8, tag=f"tao{j}",
                                     name=f"tao{j}") for j in range(2)]
                    for qp in range(nq):
                        # q*k products and block-ones score reduction
                        psc = pssc.tile([16, 1024], F32, tag="sc", name="sc")
                        for i in range(NB):
                            pr = prodp.tile([128, LBS], BF16, tag="pr", name="pr")
                            qv = tq[:, i, qp * SLAB:(qp + 1) * SLAB] \
                                .unsqueeze(1).broadcast_to([128, L, SLAB])
                            nc.vector.tensor_tensor(
                                out=pr[:].rearrange("p (a b) -> p a b", a=L),
                                in0=qv,
                                in1=tk[:, i, :].rearrange("p (a b) -> p a b", a=L),
                                op=ALU.mult)
                            nc.tensor.matmul(psc[:, 0:512], onesb[i][:],
                                             pr[:, 0:512],
                                             start=(i == 0), stop=(i == NB - 1))
                            nc.tensor.matmul(psc[:, 512:768], onesb[i][:],
                                             pr[:, 512:768],
                                             start=(i == 0), stop=(i == NB - 1))
                        e_sb = smallp.tile([16, LBS], BF16, tag="esb", name="esb")
                        nc.scalar.activation(e_sb[:], psc[:, :LBS], AF.Exp)
                        # broadcast UNNORMALIZED e to feature rows right away
                        # (DRAM round-trip; normalization folds in after the
                        # key-reduce, off the critical path)
                        esc = alscp.tile([16, LBS], BF16, tag="esc", name="esc")
                        nc.sync.dma_start(esc[:], e_sb[:])
                        pal = palp.tile([128, NB, LBS], BF16, tag="pal", name="pal")
                        for ih in range(2):
                            nc.sync.dma_start(
                                pal[0:64, ih * 4:ih * 4 + 4, :],
                                esc[8 * ih:8 * ih + 8:2, :]
                                    .unsqueeze(0).broadcast_to([64, 4, LBS]))
                            nc.sync.dma_start(
                                pal[64:128, ih * 4:ih * 4 + 4, :],
                                esc[8 * ih + 1:8 * ih + 8:2, :]
                                    .unsqueeze(0).broadcast_to([64, 4, LBS]))
                        # den via contiguous add tree, then 1/den broadcast
                        dt1 = smallp.tile([16, 3 * SLAB], F32, tag="dt1", name="dt1")
                        nc.vector.tensor_tensor(out=dt1[:], in0=e_sb[:, 0:384],
                                                in1=e_sb[:, 384:768], op=ALU.add)
                        dt2 = smallp.tile([16, SLAB], F32, tag="dt2", name="dt2")
                        nc.vector.tensor_tensor(out=dt2[:], in0=dt1[:, 0:SLAB],
                                                in1=dt1[:, SLAB:2 * SLAB], op=ALU.add)
                        den = smallp.tile([16, SLAB], F32, tag="den", name="den")
                        nc.vector.tensor_tensor(out=den[:], in0=dt2[:],
                                                in1=dt1[:, 2 * SLAB:3 * SLAB],
                                                op=ALU.add)
                        rden = smallp.tile([16, SLAB], BF16, tag="rden", name="rden")
                        with nc.allow_low_precision("bf16 softmax denom"):
                            nc.vector.reciprocal(rden[:], den[:])
                        rdsc = alscp.tile([16, SLAB], BF16, tag="rdsc", name="rdsc")
                        nc.sync.dma_start(rdsc[:], rden[:])
                        rpal = palp.tile([128, NB, SLAB], BF16, tag="rpal",
                                         name="rpal")
                        nc.sync.dma_start(
                            rpal[0:64, :, :],
                            rdsc[0:16:2, :].unsqueeze(0).broadcast_to([64, NB, SLAB]))
                        nc.sync.dma_start(
                            rpal[64:128, :, :],
                            rdsc[1:16:2, :].unsqueeze(0).broadcast_to([64, NB, SLAB]))
                        for j4 in range(2):
                            i0 = j4 * 4
                            avb = avbp.tile([128, 4, LBS], BF16, tag="avb",
                                            name="avb")
                            nc.vector.tensor_tensor(out=avb[:],
                                                    in0=pal[:, i0:i0 + 4, :],
                                                    in1=tv[:, i0:i0 + 4, :],
                                                    op=ALU.mult)
                            # sum over the 6 key positions: contiguous add tree
                            at1 = avbp.tile([128, 4, 3 * SLAB], BF16, tag="at1",
                                            name="at1")
                            nc.vector.tensor_tensor(out=at1[:],
                                                    in0=avb[:, :, 0:384],
                                                    in1=avb[:, :, 384:768],
                                                    op=ALU.add)
                            at2 = avbp.tile([128, 4, SLAB], BF16, tag="at2",
                                            name="at2")
                            nc.vector.tensor_tensor(out=at2[:],
                                                    in0=at1[:, :, 0:SLAB],
                                                    in1=at1[:, :, SLAB:2 * SLAB],
                                                    op=ALU.add)
                            at3 = avbp.tile([128, 4, SLAB], BF16, tag="at3",
                                            name="at3")
                            nc.vector.tensor_tensor(out=at3[:], in0=at2[:],
                                                    in1=at1[:, :, 2 * SLAB:3 * SLAB],
                                                    op=ALU.add)
                            with nc.allow_low_precision("fp8 attn-av accum"):
                                nc.vector.tensor_tensor(
                                    out=tao[j4][:, :, qp * SLAB:(qp + 1) * SLAB],
                                    in0=at3[:],
                                    in1=rpal[:, i0:i0 + 4, :], op=ALU.mult)

                    # output projection -> messages (fp8, k-subtile-major)
                    msg = msgs_n if g == 0 else msgs_v
                    for o in range(NB):
                        ps = psmm.tile([128, 1024], F32, tag="mm", name="mm")
                        for off, ln in _chunks(nqt):
                            for j in range(4):
                                nc.tensor.matmul(
                                    ps[:, off:off + ln],
                                    wo[:, 2 * j:2 * j + 2, o * 128:(o + 1) * 128],
                                    tao[j // 2][:, 2 * (j % 2):2 * (j % 2) + 2,
                                                off:off + ln],
                                    start=(j == 0), stop=(j == 3),
                                    perf_mode=DR)
                        if g == 0:
                            dst = msg[:].rearrange("p (q i) c -> p q i c", i=NB)[
                                :, 0:nq, o, slab * SLAB:(slab + 1) * SLAB]
                        else:
                            dst = msg[:, (g - 1) * NB + o,
                                      slab * SLAB:(slab + 1) * SLAB].unsqueeze(1)
                        nc.scalar.activation(
                            dst,
                            ps[:, :nqt].rearrange("p (q c) -> p q c", q=nq),
                            AF.Copy, scale=IWS)

                steps = [(g, sl) for g in (1, 2, 3, 4, 5, 0) for sl in range(NSLAB)]
                prev = None
                for idx, (g, sl) in enumerate(steps):
                    cur = emit_kvq(g, sl)
                    if prev is not None:
                        emit_attn(prev)
                    prev = cur
                emit_attn(prev)

            # ================= PASS B =================
            with tc.tile_pool(name="globb", bufs=1) as globb, \
                 tc.tile_pool(name="psmm2", bufs=4, space="PSUM") as psmm2, \
                 tc.tile_pool(name="psrow", bufs=1, space="PSUM") as psrow, \
                 tc.tile_pool(name="psbc", bufs=1, space="PSUM") as psbc:

                def aggregate(msgs, ag_dram, gates):
                    with tc.tile_pool(name="agw", bufs=1) as agw, \
                         tc.tile_pool(name="agacc", bufs=1) as agaccp:
                        acc = [agaccp.tile([128, bc], BF16, tag=f"acc{o}",
                                           name=f"acc{o}") for o in range(NB)]
                        for st in range(2):
                            agt = agw.tile([128, 20, D], FP8, tag="agt", name="agt")
                            nc.sync.dma_start(agt[:], ag_dram[:, st * 20:st * 20 + 20])
                            for o in range(NB):
                                ps = psmm2.tile([128, 512], F32, tag="mm2",
                                                name="mm2")
                                for k in range(10):
                                    nc.tensor.matmul(
                                        ps[:, :bc],
                                        agt[:, 2 * k:2 * k + 2, o * 128:(o + 1) * 128],
                                        msgs[:, st * 20 + 2 * k:st * 20 + 2 * k + 2, :],
                                        start=(k == 0), stop=(k == 9),
                                        perf_mode=DR)
                                if st == 0:
                                    with nc.allow_low_precision("agg acc bf16"):
                                        nc.scalar.copy(acc[o][:], ps[:, :bc])
                                else:
                                    tmp = agaccp.tile([128, bc], F32, tag="agtmp",
                                                      name="agtmp")
                                    nc.vector.tensor_tensor(
                                        out=tmp[:], in0=ps[:, :bc],
                                        in1=acc[o][:], op=ALU.add)
                                    nc.scalar.activation(gates[o][:], tmp[:],
                                                         AF.Sigmoid, scale=IWS)

                def center(xt, ntok, tag, cpool):
                    # mean over D via ones-matmul, broadcast, subtract in place
                    mneg = cpool.tile([1, ntok], BF16, tag=f"{tag}mn", name=f"{tag}mn")
                    for off, ln in _chunks(ntok):
                        ps = psrow.tile([1, 512], F32, tag="row", name="row")
                        for i in range(NB):
                            nc.tensor.matmul(ps[:, :ln], onescol[:],
                                             xt[i][:, off:off + ln],
                                             start=(i == 0), stop=(i == NB - 1))
                        nc.scalar.activation(mneg[:, off:off + ln], ps[:, :ln],
                                             AF.Copy, scale=-1.0)
                    for off, ln in _chunks(ntok):
                        pb = psbc.tile([128, 512], F32, tag="bc", name="bc")
                        nc.tensor.matmul(pb[:, :ln], onesrow[:],
                                         mneg[:, off:off + ln], start=True, stop=True)
                        for i in range(NB):
                            nc.vector.tensor_tensor(
                                out=xt[i][:, off:off + ln],
                                in0=xt[i][:, off:off + ln],
                                in1=pb[:, :ln], op=ALU.add)

                def ffn(xt, ntok, w1_dram, w2_dram, u, tag, w1p, w2p, hp,
                        nparts=2):
                    fpp = NF // nparts
                    if True:
                        for part in range(nparts):
                            f0 = part * fpp
                            w1t = [w1p.tile([128, fpp * 128], BF16, tag=f"w1h{i}",
                                            name=f"{tag}w1h{i}") for i in range(NB)]
                            for i in range(NB):
                                nc.sync.dma_start(
                                    w1t[i][:],
                                    w1_dram[i, :, f0 * 128:(f0 + fpp) * 128])
                            w2t = [w2p.tile([128, D], BF16, tag=f"w2h{f}",
                                            name=f"{tag}w2h{f}") for f in range(fpp)]
                            for f in range(fpp):
                                nc.sync.dma_start(w2t[f][:], w2_dram[f0 + f])
                            for off, ln in _chunks(ntok):
                                ht = [hp.tile([128, 512], BF16, tag=f"ht{f}",
                                              name=f"{tag}ht{f}") for f in range(fpp)]
                                for f in range(fpp):
                                    ps = psmm2.tile([128, 512], F32, tag="mm2",
                                                    name="mm2")
                                    for i in range(NB):
                                        nc.tensor.matmul(
                                            ps[:, :ln],
                                            w1t[i][:, f * 128:(f + 1) * 128],
                                            xt[i][:, off:off + ln],
                                            start=(i == 0), stop=(i == NB - 1))
                                    nc.scalar.activation(ht[f][:, :ln],
                                                         ps[:, :ln], AF.Relu)
                                for o in range(NB):
                                    ps = psmm2.tile([128, 512], F32, tag="mm2",
                                                    name="mm2")
                                    for f in range(fpp):
                                        nc.tensor.matmul(
                                            ps[:, :ln],
                                            w2t[f][:, o * 128:(o + 1) * 128],
                                            ht[f][:, :ln],
                                            start=(f == 0), stop=(f == fpp - 1))
                                    with nc.allow_low_precision("bf16 ffn resid"):
                                        nc.vector.tensor_tensor(
                                            out=u[o][:, off:off + ln],
                                            in0=ps[:, :ln],
                                            in1=(xt[o] if part == 0 else u[o])
                                                [:, off:off + ln],
                                            op=ALU.add)

                def layernorm_out(u, ntok, pos0, npos, tag, lnp):
                    # fully per-chunk: stats, row math, broadcast, apply, DMA
                    for off, ln in _chunks(ntok):
                        s1 = lnp.tile([1, 512], F32, tag=f"{tag}s1", name=f"{tag}s1")
                        s2 = lnp.tile([1, 512], F32, tag=f"{tag}s2", name=f"{tag}s2")
                        ps = psrow.tile([1, 512], F32, tag="row", name="row")
                        for i in range(NB):
                            nc.tensor.matmul(ps[:, :ln], onescol[:],
                                             u[i][:, off:off + ln],
                                             start=(i == 0), stop=(i == NB - 1))
                        nc.scalar.copy(s1[:, :ln], ps[:, :ln])
                        ps2 = psrow.tile([1, 512], F32, tag="row2", name="row2")
                        for i in range(NB):
                            usq = lnp.tile([128, 512], BF16, tag=f"{tag}usq",
                                           name=f"{tag}usq")
                            nc.scalar.activation(usq[:, :ln], u[i][:, off:off + ln],
                                                 AF.Square)
                            nc.tensor.matmul(ps2[:, :ln], onescol[:], usq[:, :ln],
                                             start=(i == 0), stop=(i == NB - 1))
                        nc.scalar.copy(s2[:, :ln], ps2[:, :ln])
                        # ta <- mu^2 ; s2 <- var ; ta <- sd ; tb <- 1/sd ; s1 <- mu/sd
                        ta = lnp.tile([1, 512], F32, tag=f"{tag}ta", name=f"{tag}ta")
                        tb = lnp.tile([1, 512], F32, tag=f"{tag}tb", name=f"{tag}tb")
                        nc.scalar.activation(ta[:, :ln], s1[:, :ln], AF.Square)
                        nc.vector.tensor_tensor(out=s2[:, :ln], in0=s2[:, :ln],
                                                in1=ta[:, :ln], op=ALU.subtract)
                        nc.scalar.activation(ta[:, :ln], s2[:, :ln], AF.Sqrt,
                                             bias=epst[:])
                        nc.vector.reciprocal(tb[:, :ln], ta[:, :ln])
                        nc.vector.tensor_tensor(out=s1[:, :ln], in0=s1[:, :ln],
                                                in1=tb[:, :ln], op=ALU.mult)
                        prb = psbc.tile([128, 512], F32, tag="bc", name="bc")
                        nc.tensor.matmul(prb[:, :ln], onesrow32[:],
                                         tb[:, :ln], start=True, stop=True)
                        pmb = psbc.tile([128, 512], F32, tag="bc2", name="bc2")
                        nc.tensor.matmul(pmb[:, :ln], onesrow32[:],
                                         s1[:, :ln], start=True, stop=True)
                        p0 = pos0 + off // bc
                        for i in range(NB):
                            outf = lnp.tile([128, 512], F32, tag=f"{tag}out",
                                            name=f"{tag}out")
                            nc.vector.tensor_tensor(out=outf[:, :ln],
                                                    in0=u[i][:, off:off + ln],
                                                    in1=prb[:, :ln], op=ALU.mult)
                            nc.vector.tensor_tensor(out=outf[:, :ln], in0=outf[:, :ln],
                                                    in1=pmb[:, :ln], op=ALU.subtract)
                            nc.sync.dma_start(
                                out_d[i, :, p0:p0 + ln // bc, :]
                                    .rearrange("p a b -> p (a b)"),
                                outf[:, :ln])

                # ---- gates for both paths ----
                aggregate(msgs_v, ag1_d, gates_v)
                aggregate(msgs_n, ag2_d, gates_n)

                # ---- residual inputs, centered ----
                x1 = [globb.tile([128, S * bc], BF16, tag=f"x1{i}", name=f"x1{i}")
                      for i in range(NB)]
                for i in range(NB):
                    nc.sync.dma_start(
                        x1[i][:].rearrange("p (a b) -> p a b", a=S),
                        tgt_d[:, i, 1:L])
                    nc.vector.tensor_tensor(
                        out=x1[i][:].rearrange("p (a b) -> p a b", a=S),
                        in0=x1[i][:].rearrange("p (a b) -> p a b", a=S),
                        in1=gates_v[i][:].unsqueeze(1).broadcast_to([128, S, bc]),
                        op=ALU.add)
                with tc.tile_pool(name="cpool", bufs=1) as cpool:
                    center(x1, S * bc, "c1", cpool)
                    x3 = [globb.tile([128, bc], BF16, tag=f"x3{i}", name=f"x3{i}")
                          for i in range(NB)]
                    for i in range(NB):
                        nc.sync.dma_start(x3[i][:], tgt_d[:, i, 0])
                        nc.vector.tensor_tensor(out=x3[i][:], in0=x3[i][:],
                                                in1=gates_n[i][:], op=ALU.add)
                    center(x3, bc, "c3", cpool)

                # ---- FFNs + output layernorms ----
                u1 = [globb.tile([128, S * bc], BF16, tag=f"u1{i}", name=f"u1{i}")
                      for i in range(NB)]
                u3 = [globb.tile([128, bc], BF16, tag=f"u3{i}", name=f"u3{i}")
                      for i in range(NB)]
                with tc.tile_pool(name="lnp", bufs=2) as lnp, \
                     tc.tile_pool(name="fw1", bufs=1) as fw1, \
                     tc.tile_pool(name="fw2", bufs=1) as fw2, \
                     tc.tile_pool(name="fh", bufs=1) as fh:
                    ffn(x1, S * bc, w11_d, w12_d, u1, "f1", fw1, fw2, fh)
                    layernorm_out(u1, S * bc, 1, S, "ln", lnp)
                    ffn(x3, bc, w21_d, w22_d, u3, "f2", fw1, fw2, fh)
                    layernorm_out(u3, bc, 0, 1, "ln", lnp)

    nc.compile()
    return nc


def _host_prep(features, role_embeds, weights, bc):
    NSLAB = bc // SLAB
    src = np.asarray(features, dtype=np.float32).copy()
    src[:, :, 1:, :] += np.asarray(role_embeds, dtype=np.float32)
    tgt = np.asarray(features[0], dtype=np.float32).astype(BF)   # (B, L, D)
    Btot = src.shape[1]

    w = {}
    tr = lambda a: np.ascontiguousarray(np.asarray(a, np.float32).T)

    def blk(m, nblk, scale):
        t = (tr(m) * scale).reshape(nblk, 128, -1).transpose(1, 0, 2)
        return np.clip(np.ascontiguousarray(t), -240, 240).astype(F8)

    w_in = np.asarray(weights["w_in"], np.float32)
    w["wq"] = blk(w_in[0:D], NB, WS)
    w["wk"] = blk(w_in[D:2 * D], NB, WS)
    w["wv"] = blk(w_in[2 * D:3 * D], NB, WS)
    w["wo"] = blk(weights["w_out"], NB, WS)
    w["w11"] = tr(weights["ffn1_w1"]).reshape(NB, 128, DFF).astype(BF)
    w["w12"] = tr(weights["ffn1_w2"]).reshape(NF, 128, D).astype(BF)
    w["w21"] = tr(weights["ffn2_w1"]).reshape(NB, 128, DFF).astype(BF)
    w["w22"] = tr(weights["ffn2_w2"]).reshape(NF, 128, D).astype(BF)
    w["ag1"] = blk(weights["agg1_w"], S * NB, WS)
    w["ag2"] = blk(weights["agg2_w"], S * NB, WS)

    onesb = np.zeros((NB, 128, H), np.float32)
    for i in range(NB):
        for half in range(2):
            h = 2 * i + half
            onesb[i, half * 64:(half + 1) * 64, h] = 0.125
    w["onesb"] = onesb.astype(BF)

    in_maps = []
    for c in range(Btot // bc):
        sl = slice(c * bc, (c + 1) * bc)
        s = src[:, sl]                                     # (G, bc, L, D)
        s = s.transpose(3, 0, 2, 1)                        # (D, G, L, bc)
        s = s.reshape(NB, 128, G, L, NSLAB, SLAB).transpose(1, 2, 4, 0, 3, 5)
        s = np.ascontiguousarray(s).reshape(128, G, NSLAB, NB, L * SLAB)
        s8 = s.astype(F8)
        t = tgt[sl].transpose(2, 1, 0)                     # (D, L, bc)
        t = np.ascontiguousarray(
            t.reshape(NB, 128, L, bc).transpose(1, 0, 2, 3))
        m = {"src": s8, "tgt": t}
        m.update(w)
        in_maps.append(m)
    return in_maps


def _assert_trivial(inputs):
    for k in ("b_in", "b_out", "ffn1_b1", "ffn1_b2", "ffn2_b1", "ffn2_b2",
              "agg1_b", "agg2_b", "ln1_b", "ln2_b", "ln3_b", "ln4_b"):
        assert not np.any(np.asarray(inputs[k])), f"{k} expected to be zero"
    for k in ("ln1_g", "ln2_g", "ln3_g", "ln4_g"):
        assert np.all(np.asarray(inputs[k]) == 1.0), f"{k} expected to be ones"


def kernel(**inputs):
    from concourse.bass_utils import run_bass_kernel_spmd

    _assert_trivial(inputs)
    features = np.asarray(inputs["features"], np.float32)
    role_embeds = np.asarray(inputs["role_embeds"], np.float32)
    Btot = features.shape[1]
    bc = Btot // NCORES

    key = (bc, SLAB)
    if key not in _cache:
        _cache[key] = build(bc)
    nc = _cache[key]

    in_maps = _host_prep(features, role_embeds, inputs, bc)
    res = run_bass_kernel_spmd(nc, in_maps, list(range(len(in_maps))))

    out = features.copy()
    for c in range(len(in_maps)):
        ot = np.asarray(res.results[c]["out_t"], np.float32)
        new0 = ot.reshape(D, L, bc).transpose(2, 1, 0)     # (bc, L, D)
        out[0, c * bc:(c + 1) * bc] = new0
    return out


# revision 40
# speedup vs baseline: 1.1717x; 1.1717x over previous
"""Trainium2 Bass kernel for nn_Decoder_Layer_53738630807778.

8-core data parallel over B=2048.  Feature-major on device (feature dim on
SBUF partitions, tokens on the free axis); host pre-transposes and pre-adds
role_embeds.  Big GEMMs (QKV/O projections, FFNs, aggregations) run in
fp8e4 with DoubleRow perf mode (two 128-row k-subtiles per matmul) and
fp32 PSUM; weights are pre-scaled x64 on the host (descale folds into the
PSUM->SBUF copies / activations), except the FFN second layers which stay
unscaled so the residual add can read PSUM directly.

Attention (L=6, H=16, hd=64) per (set g, batch-slab of 128):
  scores = DVE q*k elementwise -> PE block-ones matmul reduces each head's
           64 partition rows (scaled 1/8); softmax on [16 x L*128].
  alpha  -> expanded back to feature rows with a (16,128) selection matmul,
           staged to SBUF bf16 so the AV mul runs in DVE 2x mode.
  AV     = DVE mul + strided reduce over the 6 keys.

ln1/ln3 have identity affine and every bias is zero (asserted), so they
fold: LN scale-invariance + relu positive homogeneity kill the rstd factor
(ln2/ln4 renormalize); the mean is subtracted explicitly from the FFN input
(PE row-broadcast of the ones-matmul row mean).  Verb sets g=1..5 are
processed first so the noun-path aggregation overlaps g=0's attention.
"""

import sys
import numpy as np

if "/opt/trn_rl_repo" not in sys.path:
    sys.path.insert(0, "/opt/trn_rl_repo")

import ml_dtypes

BF = ml_dtypes.bfloat16
F8 = ml_dtypes.float8_e4m3

D = 1024
H = 16
DFF = 4096
S = 5
L = 6
G = 6
NCORES = 8
NB = D // 128
NF = DFF // 128
EPS = 1e-5
WS = 64.0          # fp8 weight pre-scale
SLAB = 128

_cache = {}


def _chunks(n, step=512):
    out = []
    off = 0
    while off < n:
        out.append((off, min(step, n - off)))
        off += step
    return out


def build(bc):
    import concourse.bacc as bacc
    import concourse.mybir as mybir
    import concourse.tile as tile

    F32 = mybir.dt.float32
    BF16 = mybir.dt.bfloat16
    FP8 = mybir.dt.float8e4
    AF = mybir.ActivationFunctionType
    ALU = mybir.AluOpType
    AX = mybir.AxisListType
    DR = mybir.MatmulPerfMode.DoubleRow

    NSLAB = bc // SLAB
    LBS = L * SLAB

    nc = bacc.Bacc("TRN2", target_bir_lowering=False, debug=False)

    src_d = nc.dram_tensor("src", [128, G, NSLAB, NB, LBS], FP8, kind="ExternalInput")
    tgt_d = nc.dram_tensor("tgt", [128, NB, L, bc], BF16, kind="ExternalInput")
    wq_d = nc.dram_tensor("wq", [128, NB, D], FP8, kind="ExternalInput")
    wk_d = nc.dram_tensor("wk", [128, NB, D], FP8, kind="ExternalInput")
    wv_d = nc.dram_tensor("wv", [128, NB, D], FP8, kind="ExternalInput")
    wo_d = nc.dram_tensor("wo", [128, NB, D], FP8, kind="ExternalInput")
    w11_d = nc.dram_tensor("w11", [NB, 128, DFF], BF16, kind="ExternalInput")
    w12_d = nc.dram_tensor("w12", [NF, 128, D], BF16, kind="ExternalInput")
    w21_d = nc.dram_tensor("w21", [NB, 128, DFF], BF16, kind="ExternalInput")
    w22_d = nc.dram_tensor("w22", [NF, 128, D], BF16, kind="ExternalInput")
    ag1_d = nc.dram_tensor("ag1", [128, S * NB, D], FP8, kind="ExternalInput")
    ag2_d = nc.dram_tensor("ag2", [128, S * NB, D], FP8, kind="ExternalInput")
    ones_d = nc.dram_tensor("onesb", [NB, 128, H], BF16, kind="ExternalInput")
    out_d = nc.dram_tensor("out_t", [NB, 128, L, bc], F32, kind="ExternalOutput")

    IWS = 1.0 / WS

    with tile.TileContext(nc) as tc:
        with tc.tile_pool(name="glob", bufs=1) as glob:
            msgs_v = glob.tile([128, S * NB, bc], FP8, tag="msv", name="msv")
            msgs_n = glob.tile([128, S * NB, bc], FP8, tag="msn", name="msn")
            gates_v = [glob.tile([128, bc], BF16, tag=f"gv{o}", name=f"gv{o}")
                       for o in range(NB)]
            gates_n = [glob.tile([128, bc], BF16, tag=f"gn{o}", name=f"gn{o}")
                       for o in range(NB)]
            onescol = glob.tile([128, 1], BF16, tag="onescol", name="onescol")
            onesrow = glob.tile([1, 128], BF16, tag="onesrow", name="onesrow")
            onesrow32 = glob.tile([1, 128], F32, tag="onesrow32", name="onesrow32")
            epst = glob.tile([1, 1], F32, tag="epst", name="epst")
            nc.gpsimd.memset(onescol[:], 1.0 / 1024.0)
            nc.gpsimd.memset(onesrow[:], 1.0)
            nc.gpsimd.memset(onesrow32[:], 1.0)
            nc.gpsimd.memset(epst[:], EPS)

            # ================= PASS A: attention =================
            with tc.tile_pool(name="wa", bufs=1) as wa, \
                 tc.tile_pool(name="suba", bufs=2) as suba, \
                 tc.tile_pool(name="subk", bufs=2) as subk, \
                 tc.tile_pool(name="subq", bufs=2) as subq, \
                 tc.tile_pool(name="prodp", bufs=3) as prodp, \
                 tc.tile_pool(name="palp", bufs=2) as palp, \
                 tc.tile_pool(name="avbp", bufs=1) as avbp, \
                 tc.tile_pool(name="smallp", bufs=2) as smallp, \
                 tc.tile_pool(name="alsc", bufs=2, space="DRAM") as alscp, \
                 tc.tile_pool(name="psmm", bufs=3, space="PSUM") as psmm, \
                 tc.tile_pool(name="pssc", bufs=1, space="PSUM") as pssc:

                wq = wa.tile([128, NB, D], FP8, tag="wq", name="wq")
                wk = wa.tile([128, NB, D], FP8, tag="wk", name="wk")
                wv = wa.tile([128, NB, D], FP8, tag="wv", name="wv")
                wo = wa.tile([128, NB, D], FP8, tag="wo", name="wo")
                onesb = [wa.tile([128, H], BF16, tag=f"ones{i}", name=f"ones{i}")
                         for i in range(NB)]
                nc.sync.dma_start(wq[:], wq_d[:])
                nc.sync.dma_start(wk[:], wk_d[:])
                nc.sync.dma_start(wv[:], wv_d[:])
                nc.sync.dma_start(wo[:], wo_d[:])
                for i in range(NB):
                    nc.sync.dma_start(onesb[i][:], ones_d[i])

                def proj(wt, dst, ssrc, qoff, ntok, tks=None):
                    # both N-chunks of one output block go into a single
                    # 2-bank PSUM tile -> one fused descale copy per block
                    for o in range(NB):
                        ps = psmm.tile([128, 1024], F32, tag="mm", name="mm")
                        for off, ln in _chunks(ntok):
                            for k in range(4):
                                nc.tensor.matmul(
                                    ps[:, off:off + ln],
                                    wt[:, 2 * k:2 * k + 2, o * 128:(o + 1) * 128],
                                    ssrc[:, 2 * k:2 * k + 2,
                                         qoff + off:qoff + off + ln],
                                    start=(k == 0), stop=(k == 3),
                                    perf_mode=DR)
                        out_ap = tks[o][:, 0:ntok] if tks is not None \
                            else dst[:, o, 0:ntok]
                        nc.scalar.activation(out_ap, ps[:, :ntok],
                                             AF.Copy, scale=IWS)

                def emit_kvq(g, slab):
                    ssrc = suba.tile([128, NB, LBS], FP8, tag="ssrc", name="ssrc")
                    nc.sync.dma_start(ssrc[:], src_d[:, g, slab])
                    tk = [subk.tile([128, LBS], BF16, tag=f"tk{i}", name=f"tk{i}")
                          for i in range(NB)]
                    tv = subk.tile([128, NB, LBS], BF16, tag="tv", name="tv")
                    proj(wk, None, ssrc, 0, LBS, tks=tk)
                    proj(wv, tv, ssrc, 0, LBS)
                    nq = S if g == 0 else 1
                    qoff = SLAB if g == 0 else 0
                    nqt = nq * SLAB
                    tq = [subq.tile([128, S * SLAB], BF16, tag=f"tq{i}",
                          name=f"tq{i}") for i in range(NB)]
                    proj(wq, None, ssrc, qoff, nqt, tks=tq)
                    return (g, slab, nq, nqt, tk, tv, tq)

                def emit_attn(stt):
                    g, slab, nq, nqt, tk, tv, tq = stt
                    tao = [subq.tile([128, 4, S * SLAB], FP8, tag=f"tao{j}",
                                     name=f"tao{j}") for j in range(2)]
                    for qp in range(nq):
                        # q*k products and block-ones score reduction
                        psc = pssc.tile([16, 1024], F32, tag="sc", name="sc")
                        for i in range(NB):
                            pr = prodp.tile([128, LBS], BF16, tag="pr", name="pr")
                            qv = tq[i][:, qp * SLAB:(qp + 1) * SLAB] \
                                .unsqueeze(1).broadcast_to([128, L, SLAB])
                            nc.vector.tensor_tensor(
                                out=pr[:].rearrange("p (a b) -> p a b", a=L),
                                in0=qv,
                                in1=tk[i][:].rearrange("p (a b) -> p a b", a=L),
                                op=ALU.mult)
                            nc.tensor.matmul(psc[:, 0:512], onesb[i][:],
                                             pr[:, 0:512],
                                             start=(i == 0), stop=(i == NB - 1))
                            nc.tensor.matmul(psc[:, 512:768], onesb[i][:],
                                             pr[:, 512:768],
                                             start=(i == 0), stop=(i == NB - 1))
                        e_sb = smallp.tile([16, LBS], BF16, tag="esb", name="esb")
                        nc.scalar.activation(e_sb[:], psc[:, :LBS], AF.Exp)
                        # den via contiguous add tree (avoids 1x strided reduce)
                        dt1 = smallp.tile([16, 3 * SLAB], F32, tag="dt1", name="dt1")
                        nc.vector.tensor_tensor(out=dt1[:], in0=e_sb[:, 0:384],
                                                in1=e_sb[:, 384:768], op=ALU.add)
                        dt2 = smallp.tile([16, SLAB], F32, tag="dt2", name="dt2")
                        nc.vector.tensor_tensor(out=dt2[:], in0=dt1[:, 0:SLAB],
                                                in1=dt1[:, SLAB:2 * SLAB], op=ALU.add)
                        den = smallp.tile([16, SLAB], F32, tag="den", name="den")
                        nc.vector.tensor_tensor(out=den[:], in0=dt2[:],
                                                in1=dt1[:, 2 * SLAB:3 * SLAB],
                                                op=ALU.add)
                        rden = smallp.tile([16, SLAB], F32, tag="rden", name="rden")
                        nc.vector.reciprocal(rden[:], den[:])
                        al_sb = smallp.tile([16, LBS], BF16, tag="alsb", name="alsb")
                        nc.vector.tensor_tensor(
                            out=al_sb[:].rearrange("p (a b) -> p a b", a=L),
                            in0=e_sb[:].rearrange("p (a b) -> p a b", a=L),
                            in1=rden[:].unsqueeze(1).broadcast_to([16, L, SLAB]),
                            op=ALU.mult)
                        # expand alpha to feature rows: DRAM round-trip broadcast
                        # (split 4 ways so early blocks land sooner)
                        alsc = alscp.tile([16, LBS], BF16, tag="alsc", name="alsc")
                        nc.sync.dma_start(alsc[:], al_sb[:])
                        pal = palp.tile([128, NB, LBS], BF16, tag="pal", name="pal")
                        for ih in range(2):
                            nc.sync.dma_start(
                                pal[0:64, ih * 4:ih * 4 + 4, :],
                                alsc[8 * ih:8 * ih + 8:2, :]
                                    .unsqueeze(0).broadcast_to([64, 4, LBS]))
                            nc.sync.dma_start(
                                pal[64:128, ih * 4:ih * 4 + 4, :],
                                alsc[8 * ih + 1:8 * ih + 8:2, :]
                                    .unsqueeze(0).broadcast_to([64, 4, LBS]))
                        for j4 in range(2):
                            i0 = j4 * 4
                            avb = avbp.tile([128, 4, LBS], BF16, tag="avb",
                                            name="avb")
                            nc.vector.tensor_tensor(out=avb[:],
                                                    in0=pal[:, i0:i0 + 4, :],
                                                    in1=tv[:, i0:i0 + 4, :],
                                                    op=ALU.mult)
                            # sum over the 6 key positions: contiguous add tree
                            at1 = avbp.tile([128, 4, 3 * SLAB], BF16, tag="at1",
                                            name="at1")
                            nc.vector.tensor_tensor(out=at1[:],
                                                    in0=avb[:, :, 0:384],
                                                    in1=avb[:, :, 384:768],
                                                    op=ALU.add)
                            at2 = avbp.tile([128, 4, SLAB], BF16, tag="at2",
                                            name="at2")
                            nc.vector.tensor_tensor(out=at2[:],
                                                    in0=at1[:, :, 0:SLAB],
                                                    in1=at1[:, :, SLAB:2 * SLAB],
                                                    op=ALU.add)
                            with nc.allow_low_precision("fp8 attn-av accum"):
                                nc.vector.tensor_tensor(
                                    out=tao[j4][:, :, qp * SLAB:(qp + 1) * SLAB],
                                    in0=at2[:],
                                    in1=at1[:, :, 2 * SLAB:3 * SLAB], op=ALU.add)

                    # output projection -> messages (fp8, k-subtile-major)
                    msg = msgs_n if g == 0 else msgs_v
                    for o in range(NB):
                        ps = psmm.tile([128, 1024], F32, tag="mm", name="mm")
                        for off, ln in _chunks(nqt):
                            for j in range(4):
                                nc.tensor.matmul(
                                    ps[:, off:off + ln],
                                    wo[:, 2 * j:2 * j + 2, o * 128:(o + 1) * 128],
                                    tao[j // 2][:, 2 * (j % 2):2 * (j % 2) + 2,
                                                off:off + ln],
                                    start=(j == 0), stop=(j == 3),
                                    perf_mode=DR)
                        if g == 0:
                            dst = msg[:].rearrange("p (q i) c -> p q i c", i=NB)[
                                :, 0:nq, o, slab * SLAB:(slab + 1) * SLAB]
                        else:
                            dst = msg[:, (g - 1) * NB + o,
                                      slab * SLAB:(slab + 1) * SLAB].unsqueeze(1)
                        nc.scalar.activation(
                            dst,
                            ps[:, :nqt].rearrange("p (q c) -> p q c", q=nq),
                            AF.Copy, scale=IWS)

                steps = [(g, sl) for g in (1, 2, 3, 4, 5, 0) for sl in range(NSLAB)]
                prev = None
                for idx, (g, sl) in enumerate(steps):
                    cur = emit_kvq(g, sl)
                    if prev is not None:
                        emit_attn(prev)
                    prev = cur
                emit_attn(prev)

            # ================= PASS B =================
            with tc.tile_pool(name="globb", bufs=1) as globb, \
                 tc.tile_pool(name="psmm2", bufs=4, space="PSUM") as psmm2, \
                 tc.tile_pool(name="psrow", bufs=1, space="PSUM") as psrow, \
                 tc.tile_pool(name="psbc", bufs=1, space="PSUM") as psbc:

                def aggregate(msgs, ag_dram, gates):
                    with tc.tile_pool(name="agw", bufs=1) as agw, \
                         tc.tile_pool(name="agacc", bufs=1) as agaccp:
                        acc = [agaccp.tile([128, bc], BF16, tag=f"acc{o}",
                                           name=f"acc{o}") for o in range(NB)]
                        for st in range(2):
                            agt = agw.tile([128, 20, D], FP8, tag="agt", name="agt")
                            nc.sync.dma_start(agt[:], ag_dram[:, st * 20:st * 20 + 20])
                            for o in range(NB):
                                ps = psmm2.tile([128, 512], F32, tag="mm2",
                                                name="mm2")
                                for k in range(10):
                                    nc.tensor.matmul(
                                        ps[:, :bc],
                                        agt[:, 2 * k:2 * k + 2, o * 128:(o + 1) * 128],
                                        msgs[:, st * 20 + 2 * k:st * 20 + 2 * k + 2, :],
                                        start=(k == 0), stop=(k == 9),
                                        perf_mode=DR)
                                if st == 0:
                                    with nc.allow_low_precision("agg acc bf16"):
                                        nc.scalar.copy(acc[o][:], ps[:, :bc])
                                else:
                                    tmp = agaccp.tile([128, bc], F32, tag="agtmp",
                                                      name="agtmp")
                                    nc.vector.tensor_tensor(
                                        out=tmp[:], in0=ps[:, :bc],
                                        in1=acc[o][:], op=ALU.add)
                                    nc.scalar.activation(gates[o][:], tmp[:],
                                                         AF.Sigmoid, scale=IWS)

                def center(xt, ntok, tag, cpool):
                    # mean over D via ones-matmul, broadcast, subtract in place
                    mneg = cpool.tile([1, ntok], BF16, tag=f"{tag}mn", name=f"{tag}mn")
                    for off, ln in _chunks(ntok):
                        ps = psrow.tile([1, 512], F32, tag="row", name="row")
                        for i in range(NB):
                            nc.tensor.matmul(ps[:, :ln], onescol[:],
                                             xt[i][:, off:off + ln],
                                             start=(i == 0), stop=(i == NB - 1))
                        nc.scalar.activation(mneg[:, off:off + ln], ps[:, :ln],
                                             AF.Copy, scale=-1.0)
                    for off, ln in _chunks(ntok):
                        pb = psbc.tile([128, 512], F32, tag="bc", name="bc")
                        nc.tensor.matmul(pb[:, :ln], onesrow[:],
                                         mneg[:, off:off + ln], start=True, stop=True)
                        for i in range(NB):
                            nc.vector.tensor_tensor(
                                out=xt[i][:, off:off + ln],
                                in0=xt[i][:, off:off + ln],
                                in1=pb[:, :ln], op=ALU.add)

                def ffn(xt, ntok, w1_dram, w2_dram, u, tag, w1p, w2p, hp,
                        nparts=2):
                    fpp = NF // nparts
                    if True:
                        for part in range(nparts):
                            f0 = part * fpp
                            w1t = [w1p.tile([128, fpp * 128], BF16, tag=f"w1h{i}",
                                            name=f"{tag}w1h{i}") for i in range(NB)]
                            for i in range(NB):
                                nc.sync.dma_start(
                                    w1t[i][:],
                                    w1_dram[i, :, f0 * 128:(f0 + fpp) * 128])
                            w2t = [w2p.tile([128, D], BF16, tag=f"w2h{f}",
                                            name=f"{tag}w2h{f}") for f in range(fpp)]
                            for f in range(fpp):
                                nc.sync.dma_start(w2t[f][:], w2_dram[f0 + f])
                            for off, ln in _chunks(ntok):
                                ht = [hp.tile([128, 512], BF16, tag=f"ht{f}",
                                              name=f"{tag}ht{f}") for f in range(fpp)]
                                for f in range(fpp):
                                    ps = psmm2.tile([128, 512], F32, tag="mm2",
                                                    name="mm2")
                                    for i in range(NB):
                                        nc.tensor.matmul(
                                            ps[:, :ln],
                                            w1t[i][:, f * 128:(f + 1) * 128],
                                            xt[i][:, off:off + ln],
                                            start=(i == 0), stop=(i == NB - 1))
                                    nc.scalar.activation(ht[f][:, :ln],
                                                         ps[:, :ln], AF.Relu)
                                for o in range(NB):
                                    ps = psmm2.tile([128, 512], F32, tag="mm2",
                                                    name="mm2")
                                    for f in range(fpp):
                                        nc.tensor.matmul(
                                            ps[:, :ln],
                                            w2t[f][:, o * 128:(o + 1) * 128],
                                            ht[f][:, :ln],
                                            start=(f == 0), stop=(f == fpp - 1))
                                    with nc.allow_low_precision("bf16 ffn resid"):
                                        nc.vector.tensor_tensor(
                                            out=u[o][:, off:off + ln],
                                            in0=ps[:, :ln],
                                            in1=(xt[o] if part == 0 else u[o])
                                                [:, off:off + ln],
                                            op=ALU.add)

                def layernorm_out(u, ntok, pos0, npos, tag, lnp):
                    # fully per-chunk: stats, row math, broadcast, apply, DMA
                    for off, ln in _chunks(ntok):
                        s1 = lnp.tile([1, 512], F32, tag=f"{tag}s1", name=f"{tag}s1")
                        s2 = lnp.tile([1, 512], F32, tag=f"{tag}s2", name=f"{tag}s2")
                        ps = psrow.tile([1, 512], F32, tag="row", name="row")
                        for i in range(NB):
                            nc.tensor.matmul(ps[:, :ln], onescol[:],
                                             u[i][:, off:off + ln],
                                             start=(i == 0), stop=(i == NB - 1))
                        nc.scalar.copy(s1[:, :ln], ps[:, :ln])
                        ps2 = psrow.tile([1, 512], F32, tag="row2", name="row2")
                        for i in range(NB):
                            usq = lnp.tile([128, 512], BF16, tag=f"{tag}usq",
                                           name=f"{tag}usq")
                            nc.scalar.activation(usq[:, :ln], u[i][:, off:off + ln],
                                                 AF.Square)
                            nc.tensor.matmul(ps2[:, :ln], onescol[:], usq[:, :ln],
                                             start=(i == 0), stop=(i == NB - 1))
                        nc.scalar.copy(s2[:, :ln], ps2[:, :ln])
                        # ta <- mu^2 ; s2 <- var ; ta <- sd ; tb <- 1/sd ; s1 <- mu/sd
                        ta = lnp.tile([1, 512], F32, tag=f"{tag}ta", name=f"{tag}ta")
                        tb = lnp.tile([1, 512], F32, tag=f"{tag}tb", name=f"{tag}tb")
                        nc.scalar.activation(ta[:, :ln], s1[:, :ln], AF.Square)
                        nc.vector.tensor_tensor(out=s2[:, :ln], in0=s2[:, :ln],
                                                in1=ta[:, :ln], op=ALU.subtract)
                        nc.scalar.activation(ta[:, :ln], s2[:, :ln], AF.Sqrt,
                                             bias=epst[:])
                        nc.vector.reciprocal(tb[:, :ln], ta[:, :ln])
                        nc.vector.tensor_tensor(out=s1[:, :ln], in0=s1[:, :ln],
                                                in1=tb[:, :ln], op=ALU.mult)
                        prb = psbc.tile([128, 512], F32, tag="bc", name="bc")
                        nc.tensor.matmul(prb[:, :ln], onesrow32[:],
                                         tb[:, :ln], start=True, stop=True)
                        pmb = psbc.tile([128, 512], F32, tag="bc2", name="bc2")
                        nc.tensor.matmul(pmb[:, :ln], onesrow32[:],
                                         s1[:, :ln], start=True, stop=True)
                        p0 = pos0 + off // bc
                        for i in range(NB):
                            outf = lnp.tile([128, 512], F32, tag=f"{tag}out",
                                            name=f"{tag}out")
                            nc.vector.tensor_tensor(out=outf[:, :ln],
                                                    in0=u[i][:, off:off + ln],
                                                    in1=prb[:, :ln], op=ALU.mult)
                            nc.vector.tensor_tensor(out=outf[:, :ln], in0=outf[:, :ln],
                                                    in1=pmb[:, :ln], op=ALU.subtract)
                            nc.sync.dma_start(
                                out_d[i, :, p0:p0 + ln // bc, :]
                                    .rearrange("p a b -> p (a b)"),
                                outf[:, :ln])

                # ---- gates for both paths ----
                aggregate(msgs_v, ag1_d, gates_v)
                aggregate(msgs_n, ag2_d, gates_n)

                # ---- residual inputs, centered ----
                x1 = [globb.tile([128, S * bc], BF16, tag=f"x1{i}", name=f"x1{i}")
                      for i in range(NB)]
                for i in range(NB):
                    nc.sync.dma_start(
                        x1[i][:].rearrange("p (a b) -> p a b", a=S),
                        tgt_d[:, i, 1:L])
                    nc.vector.tensor_tensor(
                        out=x1[i][:].rearrange("p (a b) -> p a b", a=S),
                        in0=x1[i][:].rearrange("p (a b) -> p a b", a=S),
                        in1=gates_v[i][:].unsqueeze(1).broadcast_to([128, S, bc]),
                        op=ALU.add)
                with tc.tile_pool(name="cpool", bufs=1) as cpool:
                    center(x1, S * bc, "c1", cpool)
                    x3 = [globb.tile([128, bc], BF16, tag=f"x3{i}", name=f"x3{i}")
                          for i in range(NB)]
                    for i in range(NB):
                        nc.sync.dma_start(x3[i][:], tgt_d[:, i, 0])
                        nc.vector.tensor_tensor(out=x3[i][:], in0=x3[i][:],
                                                in1=gates_n[i][:], op=ALU.add)
                    center(x3, bc, "c3", cpool)

                # ---- FFNs + output layernorms ----
                u1 = [globb.tile([128, S * bc], BF16, tag=f"u1{i}", name=f"u1{i}")
                      for i in range(NB)]
                u3 = [globb.tile([128, bc], BF16, tag=f"u3{i}", name=f"u3{i}")
                      for i in range(NB)]
                with tc.tile_pool(name="lnp", bufs=2) as lnp, \
                     tc.tile_pool(name="fw1", bufs=1) as fw1, \
                     tc.tile_pool(name="fw2", bufs=1) as fw2, \
                     tc.tile_pool(name="fh", bufs=1) as fh:
                    ffn(x1, S * bc, w11_d, w12_d, u1, "f1", fw1, fw2, fh)
                    layernorm_out(u1, S * bc, 1, S, "ln", lnp)
                    ffn(x3, bc, w21_d, w22_d, u3, "f2", fw1, fw2, fh)
                    layernorm_out(u3, bc, 0, 1, "ln", lnp)

    nc.compile()
    return nc


def _host_prep(features, role_embeds, weights, bc):
    NSLAB = bc // SLAB
    src = np.asarray(features, dtype=np.float32).copy()
    src[:, :, 1:, :] += np.asarray(role_embeds, dtype=np.float32)
    tgt = np.asarray(features[0], dtype=np.float32).astype(BF)   # (B, L, D)
    Btot = src.shape[1]

    w = {}
    tr = lambda a: np.ascontiguousarray(np.asarray(a, np.float32).T)

    def blk(m, nblk, scale):
        t = (tr(m) * scale).reshape(nblk, 128, -1).transpose(1, 0, 2)
        return np.clip(np.ascontiguousarray(t), -240, 240).astype(F8)

    w_in = np.asarray(weights["w_in"], np.float32)
    w["wq"] = blk(w_in[0:D], NB, WS)
    w["wk"] = blk(w_in[D:2 * D], NB, WS)
    w["wv"] = blk(w_in[2 * D:3 * D], NB, WS)
    w["wo"] = blk(weights["w_out"], NB, WS)
    w["w11"] = tr(weights["ffn1_w1"]).reshape(NB, 128, DFF).astype(BF)
    w["w12"] = tr(weights["ffn1_w2"]).reshape(NF, 128, D).astype(BF)
    w["w21"] = tr(weights["ffn2_w1"]).reshape(NB, 128, DFF).astype(BF)
    w["w22"] = tr(weights["ffn2_w2"]).reshape(NF, 128, D).astype(BF)
    w["ag1"] = blk(weights["agg1_w"], S * NB, WS)
    w["ag2"] = blk(weights["agg2_w"], S * NB, WS)

    onesb = np.zeros((NB, 128, H), np.float32)
    for i in range(NB):
        for half in range(2):
            h = 2 * i + half
            onesb[i, half * 64:(half + 1) * 64, h] = 0.125
    w["onesb"] = onesb.astype(BF)

    in_maps = []
    for c in range(Btot // bc):
        sl = slice(c * bc, (c + 1) * bc)
        s = src[:, sl]                                     # (G, bc, L, D)
        s = s.transpose(3, 0, 2, 1)                        # (D, G, L, bc)
        s = s.reshape(NB, 128, G, L, NSLAB, SLAB).transpose(1, 2, 4, 0, 3, 5)
        s = np.ascontiguousarray(s).reshape(128, G, NSLAB, NB, L * SLAB)
        s8 = s.astype(F8)
        t = tgt[sl].transpose(2, 1, 0)                     # (D, L, bc)
        t = np.ascontiguousarray(
            t.reshape(NB, 128, L, bc).transpose(1, 0, 2, 3))
        m = {"src": s8, "tgt": t}
        m.update(w)
        in_maps.append(m)
    return in_maps


def _assert_trivial(inputs):
    for k in ("b_in", "b_out", "ffn1_b1", "ffn1_b2", "ffn2_b1", "ffn2_b2",
              "agg1_b", "agg2_b", "ln1_b", "ln2_b", "ln3_b", "ln4_b"):
        assert not np.any(np.asarray(inputs[k])), f"{k} expected to be zero"
    for k in ("ln1_g", "ln2_g", "ln3_g", "ln4_g"):
        assert np.all(np.asarray(inputs[k]) == 1.0), f"{k} expected to be ones"


def kernel(**inputs):
    from concourse.bass_utils import run_bass_kernel_spmd

    _assert_trivial(inputs)
    features = np.asarray(inputs["features"], np.float32)
    role_embeds = np.asarray(inputs["role_embeds"], np.float32)
    Btot = features.shape[1]
    bc = Btot // NCORES

    key = (bc, SLAB)
    if key not in _cache:
        _cache[key] = build(bc)
    nc = _cache[key]

    in_maps = _host_prep(features, role_embeds, inputs, bc)
    res = run_bass_kernel_spmd(nc, in_maps, list(range(len(in_maps))))

    out = features.copy()
    for c in range(len(in_maps)):
        ot = np.asarray(res.results[c]["out_t"], np.float32)
        new0 = ot.reshape(D, L, bc).transpose(2, 1, 0)     # (bc, L, D)
        out[0, c * bc:(c + 1) * bc] = new0
    return out


# revision 52
# speedup vs baseline: 1.2493x; 1.0662x over previous
"""Trainium2 Bass kernel for nn_Decoder_Layer_53738630807778.

8-core data parallel over B=2048.  Feature-major on device (feature dim on
SBUF partitions, tokens on the free axis); host pre-transposes and pre-adds
role_embeds.  Big GEMMs (QKV/O projections, FFNs, aggregations) run in
fp8e4 with DoubleRow perf mode (two 128-row k-subtiles per matmul) and
fp32 PSUM; weights are pre-scaled x64 on the host (descale folds into the
PSUM->SBUF copies / activations), except the FFN second layers which stay
unscaled so the residual add can read PSUM directly.

Attention (L=6, H=16, hd=64) per (set g, batch-slab of 128):
  scores = DVE q*k elementwise -> PE block-ones matmul reduces each head's
           64 partition rows (scaled 1/8); softmax on [16 x L*128].
  alpha  -> expanded back to feature rows with a (16,128) selection matmul,
           staged to SBUF bf16 so the AV mul runs in DVE 2x mode.
  AV     = DVE mul + strided reduce over the 6 keys.

ln1/ln3 have identity affine and every bias is zero (asserted), so they
fold: LN scale-invariance + relu positive homogeneity kill the rstd factor
(ln2/ln4 renormalize); the mean is subtracted explicitly from the FFN input
(PE row-broadcast of the ones-matmul row mean).  Verb sets g=1..5 are
processed first so the noun-path aggregation overlaps g=0's attention.
"""

import sys
import numpy as np

if "/opt/trn_rl_repo" not in sys.path:
    sys.path.insert(0, "/opt/trn_rl_repo")

import ml_dtypes

BF = ml_dtypes.bfloat16
F8 = ml_dtypes.float8_e4m3

D = 1024
H = 16
DFF = 4096
S = 5
L = 6
G = 6
NCORES = 8
NB = D // 128
NF = DFF // 128
EPS = 1e-5
WS = 64.0          # fp8 weight pre-scale
SLAB = 128

_cache = {}


def _chunks(n, step=512):
    out = []
    off = 0
    while off < n:
        out.append((off, min(step, n - off)))
        off += step
    return out


def build(bc):
    import concourse.bacc as bacc
    import concourse.mybir as mybir
    import concourse.tile as tile

    F32 = mybir.dt.float32
    BF16 = mybir.dt.bfloat16
    FP8 = mybir.dt.float8e4
    AF = mybir.ActivationFunctionType
    ALU = mybir.AluOpType
    AX = mybir.AxisListType
    DR = mybir.MatmulPerfMode.DoubleRow

    NSLAB = bc // SLAB
    LBS = L * SLAB

    nc = bacc.Bacc("TRN2", target_bir_lowering=False, debug=False)

    src_d = nc.dram_tensor("src", [128, G, NSLAB, NB, LBS], FP8, kind="ExternalInput")
    tgt_d = nc.dram_tensor("tgt", [128, NB, L, bc], BF16, kind="ExternalInput")
    wq_d = nc.dram_tensor("wq", [128, NB, D], FP8, kind="ExternalInput")
    wk_d = nc.dram_tensor("wk", [128, NB, D], FP8, kind="ExternalInput")
    wv_d = nc.dram_tensor("wv", [128, NB, D], FP8, kind="ExternalInput")
    wo_d = nc.dram_tensor("wo", [128, NB, D], FP8, kind="ExternalInput")
    w11_d = nc.dram_tensor("w11", [NB, 128, DFF], BF16, kind="ExternalInput")
    w12_d = nc.dram_tensor("w12", [NF, 128, D], BF16, kind="ExternalInput")
    w21_d = nc.dram_tensor("w21", [NB, 128, DFF], BF16, kind="ExternalInput")
    w22_d = nc.dram_tensor("w22", [NF, 128, D], BF16, kind="ExternalInput")
    ag1_d = nc.dram_tensor("ag1", [128, S * NB, D], FP8, kind="ExternalInput")
    ag2_d = nc.dram_tensor("ag2", [128, S * NB, D], FP8, kind="ExternalInput")
    ones_d = nc.dram_tensor("onesb", [NB, 128, H], BF16, kind="ExternalInput")
    out_d = nc.dram_tensor("out_t", [NB, 128, L, bc], F32, kind="ExternalOutput")

    IWS = 1.0 / WS

    with tile.TileContext(nc) as tc:
        with tc.tile_pool(name="glob", bufs=1) as glob:
            msgs_v = glob.tile([128, S * NB, bc], FP8, tag="msv", name="msv")
            msgs_n = glob.tile([128, S * NB, bc], FP8, tag="msn", name="msn")
            gates_v = [glob.tile([128, bc], BF16, tag=f"gv{o}", name=f"gv{o}")
                       for o in range(NB)]
            gates_n = [glob.tile([128, bc], BF16, tag=f"gn{o}", name=f"gn{o}")
                       for o in range(NB)]
            onescol = glob.tile([128, 1], BF16, tag="onescol", name="onescol")
            onesrow = glob.tile([1, 128], BF16, tag="onesrow", name="onesrow")
            onesrow32 = glob.tile([1, 128], F32, tag="onesrow32", name="onesrow32")
            epst = glob.tile([1, 1], F32, tag="epst", name="epst")
            nc.gpsimd.memset(onescol[:], 1.0 / 1024.0)
            nc.gpsimd.memset(onesrow[:], 1.0)
            nc.gpsimd.memset(onesrow32[:], 1.0)
            nc.gpsimd.memset(epst[:], EPS)

            # ================= PASS A: attention =================
            with tc.tile_pool(name="wa", bufs=1) as wa, \
                 tc.tile_pool(name="suba", bufs=2) as suba, \
                 tc.tile_pool(name="subk", bufs=2) as subk, \
                 tc.tile_pool(name="subq", bufs=2) as subq, \
                 tc.tile_pool(name="prodp", bufs=3) as prodp, \
                 tc.tile_pool(name="palp", bufs=2) as palp, \
                 tc.tile_pool(name="avbp", bufs=1) as avbp, \
                 tc.tile_pool(name="smallp", bufs=2) as smallp, \
                 tc.tile_pool(name="alsc", bufs=2, space="DRAM") as alscp, \
                 tc.tile_pool(name="psmm", bufs=2, space="PSUM") as psmm, \
                 tc.tile_pool(name="pssc", bufs=2, space="PSUM") as pssc:

                wq = wa.tile([128, NB, D], FP8, tag="wq", name="wq")
                wk = wa.tile([128, NB, D], FP8, tag="wk", name="wk")
                wv = wa.tile([128, NB, D], FP8, tag="wv", name="wv")
                wo = wa.tile([128, NB, D], FP8, tag="wo", name="wo")
                onesb = [wa.tile([128, H], BF16, tag=f"ones{i}", name=f"ones{i}")
                         for i in range(NB)]
                nc.sync.dma_start(wq[:], wq_d[:])
                nc.sync.dma_start(wk[:], wk_d[:])
                nc.sync.dma_start(wv[:], wv_d[:])
                nc.sync.dma_start(wo[:], wo_d[:])
                for i in range(NB):
                    nc.sync.dma_start(onesb[i][:], ones_d[i])

                def proj(wt, dst, ssrc, qoff, ntok, tks=None):
                    # both N-chunks of one output block go into a single
                    # 2-bank PSUM tile -> one fused descale copy per block
                    for o in range(NB):
                        ps = psmm.tile([128, 1024], F32, tag="mm", name="mm")
                        for off, ln in _chunks(ntok):
                            for k in range(4):
                                nc.tensor.matmul(
                                    ps[:, off:off + ln],
                                    wt[:, 2 * k:2 * k + 2, o * 128:(o + 1) * 128],
                                    ssrc[:, 2 * k:2 * k + 2,
                                         qoff + off:qoff + off + ln],
                                    start=(k == 0), stop=(k == 3),
                                    perf_mode=DR)
                        out_ap = tks[o][:, 0:ntok] if tks is not None \
                            else dst[:, o, 0:ntok]
                        nc.scalar.activation(out_ap, ps[:, :ntok],
                                             AF.Copy, scale=IWS)

                def emit_kvq(g, slab):
                    ssrc = suba.tile([128, NB, LBS], FP8, tag="ssrc", name="ssrc")
                    nc.sync.dma_start(ssrc[:], src_d[:, g, slab])
                    tk = [subk.tile([128, LBS], BF16, tag=f"tk{i}", name=f"tk{i}")
                          for i in range(NB)]
                    tv = subk.tile([128, NB, LBS], BF16, tag="tv", name="tv")
                    proj(wk, None, ssrc, 0, LBS, tks=tk)
                    proj(wv, tv, ssrc, 0, LBS)
                    nq = S if g == 0 else 1
                    qoff = SLAB if g == 0 else 0
                    nqt = nq * SLAB
                    tq = [subq.tile([128, S * SLAB], BF16, tag=f"tq{i}",
                          name=f"tq{i}") for i in range(NB)]
                    proj(wq, None, ssrc, qoff, nqt, tks=tq)
                    return (g, slab, nq, nqt, tk, tv, tq)

                def emit_attn(stt):
                    g, slab, nq, nqt, tk, tv, tq = stt
                    tao = [subq.tile([128, 4, S * SLAB], FP8, tag=f"tao{j}",
                                     name=f"tao{j}") for j in range(2)]
                    for qp in range(nq):
                        # q*k products and block-ones score reduction
                        psc = pssc.tile([16, 1024], F32, tag="sc", name="sc")
                        for i in range(NB):
                            pr = prodp.tile([128, LBS], BF16, tag="pr", name="pr")
                            qv = tq[i][:, qp * SLAB:(qp + 1) * SLAB] \
                                .unsqueeze(1).broadcast_to([128, L, SLAB])
                            nc.vector.tensor_tensor(
                                out=pr[:].rearrange("p (a b) -> p a b", a=L),
                                in0=qv,
                                in1=tk[i][:].rearrange("p (a b) -> p a b", a=L),
                                op=ALU.mult)
                            nc.tensor.matmul(psc[:, 0:512], onesb[i][:],
                                             pr[:, 0:512],
                                             start=(i == 0), stop=(i == NB - 1))
                            nc.tensor.matmul(psc[:, 512:768], onesb[i][:],
                                             pr[:, 512:768],
                                             start=(i == 0), stop=(i == NB - 1))
                        e_sb = smallp.tile([16, LBS], BF16, tag="esb", name="esb")
                        nc.scalar.activation(e_sb[:], psc[:, :LBS], AF.Exp)
                        # den via contiguous add tree (avoids 1x strided reduce)
                        dt1 = smallp.tile([16, 3 * SLAB], F32, tag="dt1", name="dt1")
                        nc.vector.tensor_tensor(out=dt1[:], in0=e_sb[:, 0:384],
                                                in1=e_sb[:, 384:768], op=ALU.add)
                        dt2 = smallp.tile([16, SLAB], F32, tag="dt2", name="dt2")
                        nc.vector.tensor_tensor(out=dt2[:], in0=dt1[:, 0:SLAB],
                                                in1=dt1[:, SLAB:2 * SLAB], op=ALU.add)
                        den = smallp.tile([16, SLAB], F32, tag="den", name="den")
                        nc.vector.tensor_tensor(out=den[:], in0=dt2[:],
                                                in1=dt1[:, 2 * SLAB:3 * SLAB],
                                                op=ALU.add)
                        rden = smallp.tile([16, SLAB], F32, tag="rden", name="rden")
                        nc.vector.reciprocal(rden[:], den[:])
                        al_sb = smallp.tile([16, LBS], BF16, tag="alsb", name="alsb")
                        nc.vector.tensor_tensor(
                            out=al_sb[:].rearrange("p (a b) -> p a b", a=L),
                            in0=e_sb[:].rearrange("p (a b) -> p a b", a=L),
                            in1=rden[:].unsqueeze(1).broadcast_to([16, L, SLAB]),
                            op=ALU.mult)
                        # expand alpha to feature rows: DRAM round-trip broadcast
                        # (split 4 ways so early blocks land sooner)
                        alsc = alscp.tile([16, LBS], BF16, tag="alsc", name="alsc")
                        nc.sync.dma_start(alsc[:], al_sb[:])
                        pal = palp.tile([128, NB, LBS], BF16, tag="pal", name="pal")
                        for ih in range(2):
                            nc.sync.dma_start(
                                pal[0:64, ih * 4:ih * 4 + 4, :],
                                alsc[8 * ih:8 * ih + 8:2, :]
                                    .unsqueeze(0).broadcast_to([64, 4, LBS]))
                            nc.sync.dma_start(
                                pal[64:128, ih * 4:ih * 4 + 4, :],
                                alsc[8 * ih + 1:8 * ih + 8:2, :]
                                    .unsqueeze(0).broadcast_to([64, 4, LBS]))
                        for j4 in range(2):
                            i0 = j4 * 4
                            avb = avbp.tile([128, 4, LBS], BF16, tag="avb",
                                            name="avb")
                            nc.vector.tensor_tensor(out=avb[:],
                                                    in0=pal[:, i0:i0 + 4, :],
                                                    in1=tv[:, i0:i0 + 4, :],
                                                    op=ALU.mult)
                            # sum over the 6 key positions: contiguous add tree
                            at1 = avbp.tile([128, 4, 3 * SLAB], BF16, tag="at1",
                                            name="at1")
                            nc.vector.tensor_tensor(out=at1[:],
                                                    in0=avb[:, :, 0:384],
                                                    in1=avb[:, :, 384:768],
                                                    op=ALU.add)
                            at2 = avbp.tile([128, 4, SLAB], BF16, tag="at2",
                                            name="at2")
                            nc.vector.tensor_tensor(out=at2[:],
                                                    in0=at1[:, :, 0:SLAB],
                                                    in1=at1[:, :, SLAB:2 * SLAB],
                                                    op=ALU.add)
                            with nc.allow_low_precision("fp8 attn-av accum"):
                                nc.vector.tensor_tensor(
                                    out=tao[j4][:, :, qp * SLAB:(qp + 1) * SLAB],
                                    in0=at2[:],
                                    in1=at1[:, :, 2 * SLAB:3 * SLAB], op=ALU.add)

                    # output projection -> messages (fp8, k-subtile-major)
                    msg = msgs_n if g == 0 else msgs_v
                    for o in range(NB):
                        ps = psmm.tile([128, 1024], F32, tag="mm", name="mm")
                        for off, ln in _chunks(nqt):
                            for j in range(4):
                                nc.tensor.matmul(
                                    ps[:, off:off + ln],
                                    wo[:, 2 * j:2 * j + 2, o * 128:(o + 1) * 128],
                                    tao[j // 2][:, 2 * (j % 2):2 * (j % 2) + 2,
                                                off:off + ln],
                                    start=(j == 0), stop=(j == 3),
                                    perf_mode=DR)
                        if g == 0:
                            dst = msg[:].rearrange("p (q i) c -> p q i c", i=NB)[
                                :, 0:nq, o, slab * SLAB:(slab + 1) * SLAB]
                        else:
                            dst = msg[:, (g - 1) * NB + o,
                                      slab * SLAB:(slab + 1) * SLAB].unsqueeze(1)
                        nc.scalar.activation(
                            dst,
                            ps[:, :nqt].rearrange("p (q c) -> p q c", q=nq),
                            AF.Copy, scale=IWS)

                steps = [(g, sl) for g in (1, 2, 3, 4, 5, 0) for sl in range(NSLAB)]
                prev = None
                for idx, (g, sl) in enumerate(steps):
                    cur = emit_kvq(g, sl)
                    if prev is not None:
                        emit_attn(prev)
                    prev = cur
                emit_attn(prev)

            # ================= PASS B =================
            with tc.tile_pool(name="globb", bufs=1) as globb, \
                 tc.tile_pool(name="psmm2", bufs=4, space="PSUM") as psmm2, \
                 tc.tile_pool(name="psrow", bufs=1, space="PSUM") as psrow, \
                 tc.tile_pool(name="psbc", bufs=1, space="PSUM") as psbc:

                def aggregate(msgs, ag_dram, gates):
                    with tc.tile_pool(name="agw", bufs=1) as agw, \
                         tc.tile_pool(name="agacc", bufs=1) as agaccp:
                        acc = [agaccp.tile([128, bc], BF16, tag=f"acc{o}",
                                           name=f"acc{o}") for o in range(NB)]
                        for st in range(2):
                            agt = agw.tile([128, 20, D], FP8, tag="agt", name="agt")
                            nc.sync.dma_start(agt[:], ag_dram[:, st * 20:st * 20 + 20])
                            for o in range(NB):
                                ps = psmm2.tile([128, 512], F32, tag="mm2",
                                                name="mm2")
                                for k in range(10):
                                    nc.tensor.matmul(
                                        ps[:, :bc],
                                        agt[:, 2 * k:2 * k + 2, o * 128:(o + 1) * 128],
                                        msgs[:, st * 20 + 2 * k:st * 20 + 2 * k + 2, :],
                                        start=(k == 0), stop=(k == 9),
                                        perf_mode=DR)
                                if st == 0:
                                    with nc.allow_low_precision("agg acc bf16"):
                                        nc.scalar.copy(acc[o][:], ps[:, :bc])
                                else:
                                    tmp = agaccp.tile([128, bc], F32, tag="agtmp",
                                                      name="agtmp")
                                    nc.vector.tensor_tensor(
                                        out=tmp[:], in0=ps[:, :bc],
                                        in1=acc[o][:], op=ALU.add)
                                    nc.scalar.activation(gates[o][:], tmp[:],
                                                         AF.Sigmoid, scale=IWS)

                def center(xt, ntok, tag, cpool):
                    # mean over D via ones-matmul, broadcast, subtract in place
                    mneg = cpool.tile([1, ntok], BF16, tag=f"{tag}mn", name=f"{tag}mn")
                    for off, ln in _chunks(ntok):
                        ps = psrow.tile([1, 512], F32, tag="row", name="row")
                        for i in range(NB):
                            nc.tensor.matmul(ps[:, :ln], onescol[:],
                                             xt[i][:, off:off + ln],
                                             start=(i == 0), stop=(i == NB - 1))
                        nc.scalar.activation(mneg[:, off:off + ln], ps[:, :ln],
                                             AF.Copy, scale=-1.0)
                    for off, ln in _chunks(ntok):
                        pb = psbc.tile([128, 512], F32, tag="bc", name="bc")
                        nc.tensor.matmul(pb[:, :ln], onesrow[:],
                                         mneg[:, off:off + ln], start=True, stop=True)
                        for i in range(NB):
                            nc.vector.tensor_tensor(
                                out=xt[i][:, off:off + ln],
                                in0=xt[i][:, off:off + ln],
                                in1=pb[:, :ln], op=ALU.add)

                def ffn(xt, xc8, ntok, w1_dram, w2_dram, u, tag, w1p, w2p, hp,
                        nparts=2):
                    fpp = NF // nparts
                    if True:
                        for part in range(nparts):
                            f0 = part * fpp
                            w1t = [w1p.tile([128, fpp * 128], BF16, tag=f"w1h{i}",
                                            name=f"{tag}w1h{i}") for i in range(NB)]
                            for i in range(NB):
                                nc.sync.dma_start(
                                    w1t[i][:],
                                    w1_dram[i, :, f0 * 128:(f0 + fpp) * 128])
                            w2t = [w2p.tile([128, D], BF16, tag=f"w2h{f}",
                                            name=f"{tag}w2h{f}") for f in range(fpp)]
                            for f in range(fpp):
                                nc.sync.dma_start(w2t[f][:], w2_dram[f0 + f])
                            for off, ln in _chunks(ntok):
                                ht = [hp.tile([128, 512], BF16, tag=f"ht{f}",
                                              name=f"{tag}ht{f}") for f in range(fpp)]
                                for f in range(fpp):
                                    ps = psmm2.tile([128, 512], F32, tag="mm2",
                                                    name="mm2")
                                    for i in range(NB):
                                        nc.tensor.matmul(
                                            ps[:, :ln],
                                            w1t[i][:, f * 128:(f + 1) * 128],
                                            xt[i][:, off:off + ln],
                                            start=(i == 0), stop=(i == NB - 1))
                                    nc.scalar.activation(ht[f][:, :ln],
                                                         ps[:, :ln], AF.Relu)
                                for o in range(NB):
                                    ps = psmm2.tile([128, 512], F32, tag="mm2",
                                                    name="mm2")
                                    for f in range(fpp):
                                        nc.tensor.matmul(
                                            ps[:, :ln],
                                            w2t[f][:, o * 128:(o + 1) * 128],
                                            ht[f][:, :ln],
                                            start=(f == 0), stop=(f == fpp - 1))
                                    with nc.allow_low_precision("bf16 ffn resid"):
                                        nc.vector.tensor_tensor(
                                            out=u[o][:, off:off + ln],
                                            in0=ps[:, :ln],
                                            in1=(xt[o] if part == 0 else u[o])
                                                [:, off:off + ln],
                                            op=ALU.add)

                def layernorm_out(u, ntok, pos0, npos, tag, lnp):
                    # fully per-chunk: stats, row math, broadcast, apply, DMA
                    for off, ln in _chunks(ntok):
                        s1 = lnp.tile([1, 512], F32, tag=f"{tag}s1", name=f"{tag}s1")
                        s2 = lnp.tile([1, 512], F32, tag=f"{tag}s2", name=f"{tag}s2")
                        ps = psrow.tile([1, 512], F32, tag="row", name="row")
                        for i in range(NB):
                            nc.tensor.matmul(ps[:, :ln], onescol[:],
                                             u[i][:, off:off + ln],
                                             start=(i == 0), stop=(i == NB - 1))
                        nc.scalar.copy(s1[:, :ln], ps[:, :ln])
                        ps2 = psrow.tile([1, 512], F32, tag="row2", name="row2")
                        for i in range(NB):
                            usq = lnp.tile([128, 512], BF16, tag=f"{tag}usq",
                                           name=f"{tag}usq")
                            nc.scalar.activation(usq[:, :ln], u[i][:, off:off + ln],
                                                 AF.Square)
                            nc.tensor.matmul(ps2[:, :ln], onescol[:], usq[:, :ln],
                                             start=(i == 0), stop=(i == NB - 1))
                        nc.scalar.copy(s2[:, :ln], ps2[:, :ln])
                        # ta <- mu^2 ; s2 <- var ; ta <- sd ; tb <- 1/sd ; s1 <- mu/sd
                        ta = lnp.tile([1, 512], F32, tag=f"{tag}ta", name=f"{tag}ta")
                        tb = lnp.tile([1, 512], F32, tag=f"{tag}tb", name=f"{tag}tb")
                        nc.scalar.activation(ta[:, :ln], s1[:, :ln], AF.Square)
                        nc.vector.tensor_tensor(out=s2[:, :ln], in0=s2[:, :ln],
                                                in1=ta[:, :ln], op=ALU.subtract)
                        nc.scalar.activation(ta[:, :ln], s2[:, :ln], AF.Sqrt,
                                             bias=epst[:])
                        nc.vector.reciprocal(tb[:, :ln], ta[:, :ln])
                        nc.vector.tensor_tensor(out=s1[:, :ln], in0=s1[:, :ln],
                                                in1=tb[:, :ln], op=ALU.mult)
                        prb = psbc.tile([128, 512], F32, tag="bc", name="bc")
                        nc.tensor.matmul(prb[:, :ln], onesrow32[:],
                                         tb[:, :ln], start=True, stop=True)
                        pmb = psbc.tile([128, 512], F32, tag="bc2", name="bc2")
                        nc.tensor.matmul(pmb[:, :ln], onesrow32[:],
                                         s1[:, :ln], start=True, stop=True)
                        p0 = pos0 + off // bc
                        for i in range(NB):
                            outf = lnp.tile([128, 512], F32, tag=f"{tag}out",
                                            name=f"{tag}out")
                            nc.vector.tensor_tensor(out=outf[:, :ln],
                                                    in0=u[i][:, off:off + ln],
                                                    in1=prb[:, :ln], op=ALU.mult)
                            nc.vector.tensor_tensor(out=outf[:, :ln], in0=outf[:, :ln],
                                                    in1=pmb[:, :ln], op=ALU.subtract)
                            nc.sync.dma_start(
                                out_d[i, :, p0:p0 + ln // bc, :]
                                    .rearrange("p a b -> p (a b)"),
                                outf[:, :ln])

                # ---- gates for both paths ----
                aggregate(msgs_v, ag1_d, gates_v)
                aggregate(msgs_n, ag2_d, gates_n)

                # ---- residual inputs, centered ----
                x1 = [globb.tile([128, S * bc], BF16, tag=f"x1{i}", name=f"x1{i}")
                      for i in range(NB)]
                for i in range(NB):
                    nc.sync.dma_start(
                        x1[i][:].rearrange("p (a b) -> p a b", a=S),
                        tgt_d[:, i, 1:L])
                    nc.vector.tensor_tensor(
                        out=x1[i][:].rearrange("p (a b) -> p a b", a=S),
                        in0=x1[i][:].rearrange("p (a b) -> p a b", a=S),
                        in1=gates_v[i][:].unsqueeze(1).broadcast_to([128, S, bc]),
                        op=ALU.add)
                with tc.tile_pool(name="cpool", bufs=1) as cpool:
                    center(x1, S * bc, "c1", cpool)
                    x3 = [globb.tile([128, bc], BF16, tag=f"x3{i}", name=f"x3{i}")
                          for i in range(NB)]
                    for i in range(NB):
                        nc.sync.dma_start(x3[i][:], tgt_d[:, i, 0])
                        nc.vector.tensor_tensor(out=x3[i][:], in0=x3[i][:],
                                                in1=gates_n[i][:], op=ALU.add)
                    center(x3, bc, "c3", cpool)

                # ---- FFNs + output layernorms ----
                u1 = [globb.tile([128, S * bc], BF16, tag=f"u1{i}", name=f"u1{i}")
                      for i in range(NB)]
                u3 = [globb.tile([128, bc], BF16, tag=f"u3{i}", name=f"u3{i}")
                      for i in range(NB)]
                with tc.tile_pool(name="lnp", bufs=2) as lnp, \
                     tc.tile_pool(name="fw1", bufs=1) as fw1, \
                     tc.tile_pool(name="fw2", bufs=1) as fw2, \
                     tc.tile_pool(name="fh", bufs=1) as fh:
                    ffn(x1, None, S * bc, w11_d, w12_d, u1, "f1", fw1, fw2, fh)
                    layernorm_out(u1, S * bc, 1, S, "ln", lnp)
                    ffn(x3, None, bc, w21_d, w22_d, u3, "f2", fw1, fw2, fh)
                    layernorm_out(u3, bc, 0, 1, "ln", lnp)

    nc.compile()
    return nc


def _host_prep(features, role_embeds, weights, bc):
    NSLAB = bc // SLAB
    src = np.asarray(features, dtype=np.float32).copy()
    src[:, :, 1:, :] += np.asarray(role_embeds, dtype=np.float32)
    tgt = np.asarray(features[0], dtype=np.float32).astype(BF)   # (B, L, D)
    Btot = src.shape[1]

    w = {}
    tr = lambda a: np.ascontiguousarray(np.asarray(a, np.float32).T)

    def blk(m, nblk, scale):
        t = (tr(m) * scale).reshape(nblk, 128, -1).transpose(1, 0, 2)
        return np.clip(np.ascontiguousarray(t), -240, 240).astype(F8)

    w_in = np.asarray(weights["w_in"], np.float32)
    w["wq"] = blk(w_in[0:D], NB, WS)
    w["wk"] = blk(w_in[D:2 * D], NB, WS)
    w["wv"] = blk(w_in[2 * D:3 * D], NB, WS)
    w["wo"] = blk(weights["w_out"], NB, WS)
    w["w11"] = tr(weights["ffn1_w1"]).reshape(NB, 128, DFF).astype(BF)
    w["w12"] = tr(weights["ffn1_w2"]).reshape(NF, 128, D).astype(BF)
    w["w21"] = tr(weights["ffn2_w1"]).reshape(NB, 128, DFF).astype(BF)
    w["w22"] = tr(weights["ffn2_w2"]).reshape(NF, 128, D).astype(BF)
    w["ag1"] = blk(weights["agg1_w"], S * NB, WS)
    w["ag2"] = blk(weights["agg2_w"], S * NB, WS)

    onesb = np.zeros((NB, 128, H), np.float32)
    for i in range(NB):
        for half in range(2):
            h = 2 * i + half
            onesb[i, half * 64:(half + 1) * 64, h] = 0.125
    w["onesb"] = onesb.astype(BF)

    in_maps = []
    for c in range(Btot // bc):
        sl = slice(c * bc, (c + 1) * bc)
        s = src[:, sl]                                     # (G, bc, L, D)
        s = s.transpose(3, 0, 2, 1)                        # (D, G, L, bc)
        s = s.reshape(NB, 128, G, L, NSLAB, SLAB).transpose(1, 2, 4, 0, 3, 5)
        s = np.ascontiguousarray(s).reshape(128, G, NSLAB, NB, L * SLAB)
        s8 = s.astype(F8)
        t = tgt[sl].transpose(2, 1, 0)                     # (D, L, bc)
        t = np.ascontiguousarray(
            t.reshape(NB, 128, L, bc).transpose(1, 0, 2, 3))
        m = {"src": s8, "tgt": t}
        m.update(w)
        in_maps.append(m)
    return in_maps


def _assert_trivial(inputs):
    for k in ("b_in", "b_out", "ffn1_b1", "ffn1_b2", "ffn2_b1", "ffn2_b2",
              "agg1_b", "agg2_b", "ln1_b", "ln2_b", "ln3_b", "ln4_b"):
        assert not np.any(np.asarray(inputs[k])), f"{k} expected to be zero"
    for k in ("ln1_g", "ln2_g", "ln3_g", "ln4_g"):
        assert np.all(np.asarray(inputs[k]) == 1.0), f"{k} expected to be ones"


def kernel(**inputs):
    from concourse.bass_utils import run_bass_kernel_spmd

    _assert_trivial(inputs)
    features = np.asarray(inputs["features"], np.float32)
    role_embeds = np.asarray(inputs["role_embeds"], np.float32)
    Btot = features.shape[1]
    bc = Btot // NCORES

    key = (bc, SLAB)
    if key not in _cache:
        _cache[key] = build(bc)
    nc = _cache[key]

    in_maps = _host_prep(features, role_embeds, inputs, bc)
    res = run_bass_kernel_spmd(nc, in_maps, list(range(len(in_maps))))

    out = features.copy()
    for c in range(len(in_maps)):
        ot = np.asarray(res.results[c]["out_t"], np.float32)
        new0 = ot.reshape(D, L, bc).transpose(2, 1, 0)     # (bc, L, D)
        out[0, c * bc:(c + 1) * bc] = new0
    return out
